# revision 41
# baseline (speedup 1.0000x reference)
"""Trainium2 Bass kernel for nn_AtomEmbedding (embedding_lookup, memory-bound).

Strategy (pure data parallel over 8 NeuronCores):
  - Every table lookup is expressed as one-hot(compare) @ table on the PE.
  - The multi-class feature columns are replicated host-side into one row per
    non-default class (the "catr" blob, 50 rows per atom-group), so a single
    DVE tensor_scalar(is_equal) against a per-partition constant turns the
    DMA'd tile directly into one-hot indicators - no broadcast matmul.
  - Binary (0/1) columns ARE their own one-hot: 5 of them ride in catr (their
    eq-against-1 is the identity), the other 16 feed the matmul raw, packed
    with the bond features into one 128-row contraction ("bb" blob).
  - Two accumulating matmuls per 512-atom group pair (PE runs at the cold
    1.2 GHz clock on this part, so streamed columns are the scarce resource):
      W1'^T @ onehot[100,512] + Wb^T @ bb[128,512] -> psum[128,512].
  - Table edge semantics (element LUT default, ringsize unknown->6, ring clip)
    fold EXACTLY into weights via a delta trick + bias vector applied by the
    Scalar engine during the PSUM->SBUF copy.
  - Everything is packed 2 atom-groups deep so compute ops and DMAs use up to
    128 partitions; all streams are bf16 (integer features are exact in bf16;
    folded tables round, ~4e-3 max rel err end-to-end).
  - Host-side work is sharding + column gather/replication + transpose only.
"""

import os
import sys

sys.path.insert(0, "/opt/trn_rl_repo")
os.environ.setdefault("MYCRO_LOCAL_CACHE", "1")

import ml_dtypes
import numpy as np

import concourse.bacc as bacc
import concourse.bass as bass
import concourse.mybir as mybir
import concourse.tile as tile
from concourse.bass_utils import run_bass_kernel_spmd

F32 = mybir.dt.float32
BF16 = mybir.dt.bfloat16
NPBF16 = ml_dtypes.bfloat16
FP8 = mybir.dt.float8e4
NPFP8 = ml_dtypes.float8_e4m3

N_CORES = 8
N_TOTAL = 1_500_000
N_SHARD = N_TOTAL // N_CORES  # 187500
G = 512                       # atoms per group (one matmul free dim)
PAIR = 2 * G                  # atoms per group-pair (packed 2-deep on partitions)
N_PAIRS = 184                 # pairs per core after padding
NP = N_PAIRS * PAIR           # 188416 padded atoms per core
PAIRS_PER_CHUNK = 8
N_CHUNKS = N_PAIRS // PAIRS_PER_CHUNK  # 23
FREE = PAIRS_PER_CHUNK * G    # 4096 free-dim per chunk tile

# original 78-feature column roles
MULTI_COLS = [0, 1, 2, 3, 6, 27, 28, 29]  # z, deg, chg, hyb, hc, rs, aron, fused
# binary cols moved into the one-hot tile (eq(x,1) == x): arom, don, acc, f16, f17
MOVED_BIN = [4, 25, 26, 23, 24]
# binary cols packed with bond into the "bb" matmul: f0..f15
BIN16 = list(range(7, 23))
NBOND = 48
NOH = 45                      # one-hot class rows per group (before moved bins)
NR = NOH + len(MOVED_BIN)     # 50 catr rows per group
NBB = len(BIN16) + NBOND      # 64 bb rows per group
NOUT = 64


def build_tables(inputs):
    """Fold all embedding tables + linear weights into the device constants.

    Returns (consts dict, catr_cols): catr_cols[r] = source column of the
    original 78-wide input for replicated row r (0..NR-1).
    """
    g = {k: np.asarray(v) for k, v in inputs.items()}
    elut = g["element_lut"].astype(np.int64)
    rvals = g["ring_values"].astype(np.int64)
    e_emb = g["element_embed"].astype(np.float64)
    d_emb = g["degree_embed"].astype(np.float64)
    r_emb = g["ring_embed"].astype(np.float64)
    c_emb = g["charge_embed"].astype(np.float64)
    a_emb = g["aromatic_embed"].astype(np.float64)
    hy_emb = g["hybrid_embed"].astype(np.float64)
    hc_emb = g["hydrogen_embed"].astype(np.float64)
    ft = g["func_tables"].astype(np.float64)
    don_emb = g["h_don_embed"].astype(np.float64)
    acc_emb = g["h_acc_embed"].astype(np.float64)
    rs_emb = g["ringsize_embed"].astype(np.float64)
    an_emb = g["aroma_num_embed"].astype(np.float64)
    fi_emb = g["fused_if_embed"].astype(np.float64)
    frw = g["func_reduce_w"].astype(np.float64)
    frb = g["func_reduce_b"].astype(np.float64)
    bew = g["bond_env_w"].astype(np.float64)
    beb = g["bond_env_b"].astype(np.float64)

    def func_delta(j):
        Rj = frw[:, 2 * j:2 * j + 2]
        return (ft[j, 1] - ft[j, 0]) @ Rj.T

    # replicated one-hot rows: (source col, compare value, w_row[64])
    rows = []

    def add(col, v, cols, w):
        wr = np.zeros(NOUT)
        wr[cols[0]:cols[1]] = w
        rows.append((col, float(v), wr))

    e_def = int(np.clip(elut[0], 0, 6))
    for v in range(1, 17):
        idx = int(np.clip(elut[v], 0, 6))
        if idx != e_def:
            add(0, v, (0, 4), e_emb[idx] - e_emb[e_def])
    for k in range(1, 7):
        add(1, k, (4, 8), d_emb[k] - d_emb[0])
    for k in range(1, 8):
        add(2, k, (12, 16), c_emb[k] - c_emb[0])
    for k in range(1, 6):
        add(3, k, (20, 24), hy_emb[k] - hy_emb[0])
    for k in range(1, 5):
        add(6, k, (24, 28), hc_emb[k] - hc_emb[0])
    seen = set()
    for i in range(7):
        v = int(rvals[i])
        if v in seen:
            continue
        seen.add(v)
        w = rs_emb[i] - rs_emb[6]
        if np.any(w != 0.0):
            add(27, v, (36, 40), w)
    for k in range(1, 5):
        add(28, k, (40, 44), an_emb[k] - an_emb[0])
    for k in range(1, 8):
        add(29, k, (44, 48), fi_emb[k] - fi_emb[0])
    assert len(rows) == NOH, f"expected {NOH} one-hot rows, got {len(rows)}"

    # moved binary rows: compare value 1 -> indicator == raw value
    add(4, 1, (16, 20), a_emb[1] - a_emb[0])        # aromatic
    add(25, 1, (32, 34), don_emb[1] - don_emb[0])   # h_don
    add(26, 1, (34, 36), acc_emb[1] - acc_emb[0])   # h_acc
    add(23, 1, (28, 32), func_delta(16))            # f16
    add(24, 1, (28, 32), func_delta(17))            # f17
    assert len(rows) == NR

    catr_cols = [c for c, _, _ in rows]
    kvec = np.array([v for _, v, _ in rows])
    W1 = np.stack([w for _, _, w in rows])          # [50, 64]

    # bb-side weights: 16 func binaries then bond
    Wb1 = np.zeros((NBB, NOUT))
    for j in range(16):
        Wb1[j, 28:32] = func_delta(j)
    Wb1[16:, 48:64] = bew.T

    bias = np.zeros(NOUT)
    bias[0:4] = e_emb[e_def]
    bias[4:8] = d_emb[0]
    bias[8:12] = r_emb[1]       # ring col: clip(ring+1,0,1) == 1 always
    bias[12:16] = c_emb[0]
    bias[16:20] = a_emb[0]
    bias[20:24] = hy_emb[0]
    bias[24:28] = hc_emb[0]
    bias[28:32] = frb + sum(ft[j, 0] @ frw[:, 2 * j:2 * j + 2].T for j in range(18))
    bias[32:34] = don_emb[0]
    bias[34:36] = acc_emb[0]
    bias[36:40] = rs_emb[6]
    bias[40:44] = an_emb[0]
    bias[44:48] = fi_emb[0]
    bias[48:64] = beb

    # 2-group block-diagonal packing (bf16 matmul operands)
    kvec2 = np.tile(kvec, 2).reshape(2 * NR, 1).astype(np.float32)
    w1b = np.zeros((2 * NR, 2 * NOUT), NPBF16)
    w1b[0:NR, 0:NOUT] = W1
    w1b[NR:, NOUT:] = W1
    wbb = np.zeros((2 * NBB, 2 * NOUT), NPBF16)
    wbb[0:NBB, 0:NOUT] = Wb1
    wbb[NBB:, NOUT:] = Wb1
    bias2 = np.tile(bias, 2).reshape(2 * NOUT, 1).astype(np.float32)

    consts = {"kvec": kvec2, "w1": w1b, "wb": wbb, "bias": bias2}
    return consts, catr_cols


def build_nc(n_chunks=N_CHUNKS, pairs_per_chunk=PAIRS_PER_CHUNK):
    free = pairs_per_chunk * G
    # Bacc (not plain Bass): its compile() runs move_matmul_waits_to_ldweights
    # + generate_event_semaphores, which legalize to walrus's 1-wait-per-
    # instruction constraint on TRN2.
    nc = bacc.Bacc(None)
    catr_d = nc.dram_tensor("catr", [n_chunks, 2 * NR, free], FP8,
                            kind="ExternalInput")
    bb_d = nc.dram_tensor("bb", [n_chunks, 2 * NBB, free], BF16,
                          kind="ExternalInput")
    kvec_d = nc.dram_tensor("kvec", [2 * NR, 1], F32, kind="ExternalInput")
    w1_d = nc.dram_tensor("w1", [2 * NR, 2 * NOUT], BF16, kind="ExternalInput")
    wb_d = nc.dram_tensor("wb", [2 * NBB, 2 * NOUT], BF16, kind="ExternalInput")
    bias_d = nc.dram_tensor("bias", [2 * NOUT, 1], F32, kind="ExternalInput")
    out_d = nc.dram_tensor("out", [n_chunks, 2 * NOUT, free], BF16,
                           kind="ExternalOutput")

    with tile.TileContext(nc) as tc:
        with (
            tc.tile_pool(name="consts", bufs=1) as cpool,
            tc.tile_pool(name="catr", bufs=5) as catrp,
            tc.tile_pool(name="bb", bufs=4) as bbp,
            tc.tile_pool(name="outs", bufs=4) as outp,
            tc.tile_pool(name="oh", bufs=4) as ohp,
            tc.tile_pool(name="pso", bufs=8, space="PSUM") as pso,
        ):
            kvec_t = cpool.tile([2 * NR, 1], F32)
            nc.scalar.dma_start(kvec_t[:], kvec_d[:])
            w1_t = cpool.tile([2 * NR, 2 * NOUT], BF16)
            nc.sync.dma_start(w1_t[:], w1_d[:])
            wb_t = cpool.tile([2 * NBB, 2 * NOUT], BF16)
            nc.sync.dma_start(wb_t[:], wb_d[:])
            bias_t = cpool.tile([2 * NOUT, 1], F32)
            nc.scalar.dma_start(bias_t[:], bias_d[:])

            # superblocks of 2 chunks: one dma_start covers both chunk rows
            # (amortizes DMA fixed costs); outputs drain per chunk
            spans = []
            c = 0
            while c < n_chunks:
                s = min(2, n_chunks - c)
                spans.append((c, s))
                c += s
            for c, span in spans:
                # bb (the bigger stream) on the SP ring; catr + out on the ACT
                # ring so the two HWDGE rings generate descriptors in parallel
                catr_t = catrp.tile([2 * NR, span, free], FP8, tag="catr")
                nc.scalar.dma_start(catr_t[:],
                                    catr_d[c:c + span].rearrange("c r a -> r c a"))
                bb_t = bbp.tile([2 * NBB, span, free], BF16, tag="bb")
                nc.sync.dma_start(bb_t[:],
                                  bb_d[c:c + span].rearrange("c r a -> r c a"))
                out_t = outp.tile([2 * NOUT, span, free], BF16, tag="out")
                for j in range(span):
                    for p in range(pairs_per_chunk):
                        sl = bass.ts(p, G)
                        oh = ohp.tile([2 * NR, G], BF16)
                        nc.vector.tensor_scalar(oh[:], catr_t[:, j, sl],
                                                kvec_t[:], None,
                                                mybir.AluOpType.is_equal)
                        ops = pso.tile([2 * NOUT, G], F32)
                        nc.tensor.matmul(ops[:], w1_t[:], oh[:],
                                         start=True, stop=False)
                        nc.tensor.matmul(ops[:], wb_t[:], bb_t[:, j, sl],
                                         start=False, stop=True)
                        # bias-add + PSUM->SBUF copy on the ACT engine
                        nc.scalar.activation(out_t[:, j, sl], ops[:],
                                             mybir.ActivationFunctionType.Identity,
                                             bias=bias_t[:], scale=1.0)
                    # drain each finished chunk immediately (shorter pipeline
                    # tail than one superblock-sized out-DMA)
                    nc.scalar.dma_start(out_d[c + j], out_t[:, j])
    nc.compile()
    return nc


def shard_blobs(ai, core, catr_cols, n_chunks=N_CHUNKS,
                pairs_per_chunk=PAIRS_PER_CHUNK):
    """Slice core's shard and lay it out feature-major, 2-group packed."""
    free = pairs_per_chunk * G
    npad = n_chunks * pairs_per_chunk * PAIR
    shard = ai[core * N_SHARD:(core + 1) * N_SHARD]
    n_use = min(N_SHARD, npad)
    padded = np.zeros((npad, ai.shape[1]), np.float32)
    padded[:n_use] = shard[:n_use]
    # [chunk, pair, group, atom, col]
    v = padded.reshape(n_chunks, pairs_per_chunk, 2, G, ai.shape[1])
    catr = np.ascontiguousarray(v[..., catr_cols].transpose(0, 2, 4, 1, 3)
                                .reshape(n_chunks, 2 * NR, free)).astype(NPFP8)
    bb = np.ascontiguousarray(v[..., BIN16 + list(range(30, 78))]
                              .transpose(0, 2, 4, 1, 3)
                              .reshape(n_chunks, 2 * NBB, free)).astype(NPBF16)
    return catr, bb


def unshard_out(o, n_chunks=N_CHUNKS, pairs_per_chunk=PAIRS_PER_CHUNK):
    """[n_chunks, 128, free] device layout -> [npad, 64] atom-major."""
    npad = n_chunks * pairs_per_chunk * PAIR
    return (o.reshape(n_chunks, 2, NOUT, pairs_per_chunk, G)
            .transpose(0, 3, 1, 4, 2).reshape(npad, NOUT))


def _install_ntff_hook():
    """Register the axon NTFF profile hook that this image's antenv lacks."""
    import types
    try:
        import antenv.axon_hooks  # noqa: F401
        return
    except ImportError:
        pass
    try:
        from trn_agent_boot.trn_boot import _ntff_profile_via_ctypes
        hook = _ntff_profile_via_ctypes("/opt/axon/libaxon_pjrt.so")
        mod = types.ModuleType("antenv.axon_hooks")
        _state = {"hook": hook}
        mod.set_axon_ntff_profile_hook = lambda h: _state.__setitem__("hook", h)
        mod.get_axon_ntff_profile_hook = lambda: _state["hook"]
        sys.modules["antenv.axon_hooks"] = mod
        import antenv
        antenv.axon_hooks = mod
    except Exception as e:  # profiling is best-effort
        print(f"ntff hook install failed: {e}", file=sys.stderr)


def kernel(**inputs):
    consts, catr_cols = build_tables(inputs)
    ai = np.ascontiguousarray(np.asarray(inputs["atom_inputs"], dtype=np.float32))
    assert ai.shape == (N_TOTAL, 78), ai.shape

    in_maps = []
    for i in range(N_CORES):
        catr, bb = shard_blobs(ai, i, catr_cols)
        in_maps.append({"catr": catr, "bb": bb, **consts})

    trace = bool(int(os.environ.get("KERNEL_TRACE", "0")))
    if trace:
        _install_ntff_hook()
    nc = build_nc()
    res = run_bass_kernel_spmd(
        nc, in_maps, core_ids=list(range(N_CORES)), trace=trace,
    )
    kernel.last_result = res

    outs = []
    for i in range(N_CORES):
        o = np.asarray(res.results[i]["out"]).astype(np.float32)
        outs.append(unshard_out(o)[:N_SHARD])
    return np.ascontiguousarray(np.concatenate(outs, axis=0))


kernel.last_result = None


# revision 42
# speedup vs baseline: 1.0225x; 1.0225x over previous
"""Trainium2 Bass kernel for nn_AtomEmbedding (embedding_lookup, memory-bound).

Strategy (pure data parallel over 8 NeuronCores):
  - Every table lookup is expressed as one-hot(compare) @ table on the PE.
  - The multi-class feature columns are replicated host-side into one row per
    non-default class (the "catr" blob, 50 rows per atom-group), so a single
    DVE tensor_scalar(is_equal) against a per-partition constant turns the
    DMA'd tile directly into one-hot indicators - no broadcast matmul.
  - Binary (0/1) columns ARE their own one-hot: 5 of them ride in catr (their
    eq-against-1 is the identity), the other 16 feed the matmul raw, packed
    with the bond features into one 128-row contraction ("bb" blob).
  - Two accumulating matmuls per 512-atom group pair (PE runs at the cold
    1.2 GHz clock on this part, so streamed columns are the scarce resource):
      W1'^T @ onehot[100,512] + Wb^T @ bb[128,512] -> psum[128,512].
  - Table edge semantics (element LUT default, ringsize unknown->6, ring clip)
    fold EXACTLY into weights via a delta trick + bias vector applied by the
    Scalar engine during the PSUM->SBUF copy.
  - Everything is packed 2 atom-groups deep so compute ops and DMAs use up to
    128 partitions; all streams are bf16 (integer features are exact in bf16;
    folded tables round, ~4e-3 max rel err end-to-end).
  - Host-side work is sharding + column gather/replication + transpose only.
"""

import os
import sys

sys.path.insert(0, "/opt/trn_rl_repo")
os.environ.setdefault("MYCRO_LOCAL_CACHE", "1")

import ml_dtypes
import numpy as np

import concourse.bacc as bacc
import concourse.bass as bass
import concourse.mybir as mybir
import concourse.tile as tile
from concourse.bass_utils import run_bass_kernel_spmd

F32 = mybir.dt.float32
BF16 = mybir.dt.bfloat16
NPBF16 = ml_dtypes.bfloat16
FP8 = mybir.dt.float8e4
NPFP8 = ml_dtypes.float8_e4m3

N_CORES = 8
N_TOTAL = 1_500_000
N_SHARD = N_TOTAL // N_CORES  # 187500
G = 512                       # atoms per group (one matmul free dim)
PAIR = 2 * G                  # atoms per group-pair (packed 2-deep on partitions)
N_PAIRS = 184                 # pairs per core after padding
NP = N_PAIRS * PAIR           # 188416 padded atoms per core
PAIRS_PER_CHUNK = 8
N_CHUNKS = N_PAIRS // PAIRS_PER_CHUNK  # 23
FREE = PAIRS_PER_CHUNK * G    # 4096 free-dim per chunk tile

# original 78-feature column roles
MULTI_COLS = [0, 1, 2, 3, 6, 27, 28, 29]  # z, deg, chg, hyb, hc, rs, aron, fused
# binary cols moved into the one-hot tile (eq(x,1) == x): arom, don, acc, f16, f17
MOVED_BIN = [4, 25, 26, 23, 24]
# binary cols packed with bond into the "bb" matmul: f0..f15
BIN16 = list(range(7, 23))
NBOND = 48
NOH = 45                      # one-hot class rows per group (before moved bins)
NR = NOH + len(MOVED_BIN)     # 50 catr rows per group
NBB = len(BIN16) + NBOND      # 64 bb rows per group
NOUT = 64


def build_tables(inputs):
    """Fold all embedding tables + linear weights into the device constants.

    Returns (consts dict, catr_cols): catr_cols[r] = source column of the
    original 78-wide input for replicated row r (0..NR-1).
    """
    g = {k: np.asarray(v) for k, v in inputs.items()}
    elut = g["element_lut"].astype(np.int64)
    rvals = g["ring_values"].astype(np.int64)
    e_emb = g["element_embed"].astype(np.float64)
    d_emb = g["degree_embed"].astype(np.float64)
    r_emb = g["ring_embed"].astype(np.float64)
    c_emb = g["charge_embed"].astype(np.float64)
    a_emb = g["aromatic_embed"].astype(np.float64)
    hy_emb = g["hybrid_embed"].astype(np.float64)
    hc_emb = g["hydrogen_embed"].astype(np.float64)
    ft = g["func_tables"].astype(np.float64)
    don_emb = g["h_don_embed"].astype(np.float64)
    acc_emb = g["h_acc_embed"].astype(np.float64)
    rs_emb = g["ringsize_embed"].astype(np.float64)
    an_emb = g["aroma_num_embed"].astype(np.float64)
    fi_emb = g["fused_if_embed"].astype(np.float64)
    frw = g["func_reduce_w"].astype(np.float64)
    frb = g["func_reduce_b"].astype(np.float64)
    bew = g["bond_env_w"].astype(np.float64)
    beb = g["bond_env_b"].astype(np.float64)

    def func_delta(j):
        Rj = frw[:, 2 * j:2 * j + 2]
        return (ft[j, 1] - ft[j, 0]) @ Rj.T

    # replicated one-hot rows: (source col, compare value, w_row[64])
    rows = []

    def add(col, v, cols, w):
        wr = np.zeros(NOUT)
        wr[cols[0]:cols[1]] = w
        rows.append((col, float(v), wr))

    e_def = int(np.clip(elut[0], 0, 6))
    for v in range(1, 17):
        idx = int(np.clip(elut[v], 0, 6))
        if idx != e_def:
            add(0, v, (0, 4), e_emb[idx] - e_emb[e_def])
    for k in range(1, 7):
        add(1, k, (4, 8), d_emb[k] - d_emb[0])
    for k in range(1, 8):
        add(2, k, (12, 16), c_emb[k] - c_emb[0])
    for k in range(1, 6):
        add(3, k, (20, 24), hy_emb[k] - hy_emb[0])
    for k in range(1, 5):
        add(6, k, (24, 28), hc_emb[k] - hc_emb[0])
    seen = set()
    for i in range(7):
        v = int(rvals[i])
        if v in seen:
            continue
        seen.add(v)
        w = rs_emb[i] - rs_emb[6]
        if np.any(w != 0.0):
            add(27, v, (36, 40), w)
    for k in range(1, 5):
        add(28, k, (40, 44), an_emb[k] - an_emb[0])
    for k in range(1, 8):
        add(29, k, (44, 48), fi_emb[k] - fi_emb[0])
    assert len(rows) == NOH, f"expected {NOH} one-hot rows, got {len(rows)}"

    # moved binary rows: compare value 1 -> indicator == raw value
    add(4, 1, (16, 20), a_emb[1] - a_emb[0])        # aromatic
    add(25, 1, (32, 34), don_emb[1] - don_emb[0])   # h_don
    add(26, 1, (34, 36), acc_emb[1] - acc_emb[0])   # h_acc
    add(23, 1, (28, 32), func_delta(16))            # f16
    add(24, 1, (28, 32), func_delta(17))            # f17
    assert len(rows) == NR

    catr_cols = [c for c, _, _ in rows]
    kvec = np.array([v for _, v, _ in rows])
    W1 = np.stack([w for _, _, w in rows])          # [50, 64]

    # bb-side weights: 16 func binaries then bond
    Wb1 = np.zeros((NBB, NOUT))
    for j in range(16):
        Wb1[j, 28:32] = func_delta(j)
    Wb1[16:, 48:64] = bew.T

    bias = np.zeros(NOUT)
    bias[0:4] = e_emb[e_def]
    bias[4:8] = d_emb[0]
    bias[8:12] = r_emb[1]       # ring col: clip(ring+1,0,1) == 1 always
    bias[12:16] = c_emb[0]
    bias[16:20] = a_emb[0]
    bias[20:24] = hy_emb[0]
    bias[24:28] = hc_emb[0]
    bias[28:32] = frb + sum(ft[j, 0] @ frw[:, 2 * j:2 * j + 2].T for j in range(18))
    bias[32:34] = don_emb[0]
    bias[34:36] = acc_emb[0]
    bias[36:40] = rs_emb[6]
    bias[40:44] = an_emb[0]
    bias[44:48] = fi_emb[0]
    bias[48:64] = beb

    # 2-group block-diagonal packing (bf16 matmul operands)
    kvec2 = np.tile(kvec, 2).reshape(2 * NR, 1).astype(np.float32)
    w1b = np.zeros((2 * NR, 2 * NOUT), NPBF16)
    w1b[0:NR, 0:NOUT] = W1
    w1b[NR:, NOUT:] = W1
    wbb = np.zeros((2 * NBB, 2 * NOUT), NPBF16)
    wbb[0:NBB, 0:NOUT] = Wb1
    wbb[NBB:, NOUT:] = Wb1
    bias2 = np.tile(bias, 2).reshape(2 * NOUT, 1).astype(np.float32)

    consts = {"kvec": kvec2, "w1": w1b, "wb": wbb, "bias": bias2}
    return consts, catr_cols


def build_nc(n_chunks=N_CHUNKS, pairs_per_chunk=PAIRS_PER_CHUNK):
    free = pairs_per_chunk * G
    # Bacc (not plain Bass): its compile() runs move_matmul_waits_to_ldweights
    # + generate_event_semaphores, which legalize to walrus's 1-wait-per-
    # instruction constraint on TRN2.
    nc = bacc.Bacc(None)
    catr_d = nc.dram_tensor("catr", [n_chunks, 2 * NR, free], FP8,
                            kind="ExternalInput")
    bb_d = nc.dram_tensor("bb", [n_chunks, 2 * NBB, free], BF16,
                          kind="ExternalInput")
    kvec_d = nc.dram_tensor("kvec", [2 * NR, 1], F32, kind="ExternalInput")
    w1_d = nc.dram_tensor("w1", [2 * NR, 2 * NOUT], BF16, kind="ExternalInput")
    wb_d = nc.dram_tensor("wb", [2 * NBB, 2 * NOUT], BF16, kind="ExternalInput")
    bias_d = nc.dram_tensor("bias", [2 * NOUT, 1], F32, kind="ExternalInput")
    out_d = nc.dram_tensor("out", [n_chunks, 2 * NOUT, free], BF16,
                           kind="ExternalOutput")

    with tile.TileContext(nc) as tc:
        with (
            tc.tile_pool(name="consts", bufs=1) as cpool,
            tc.tile_pool(name="catr", bufs=5) as catrp,
            tc.tile_pool(name="bb", bufs=4) as bbp,
            tc.tile_pool(name="outs", bufs=4) as outp,
            tc.tile_pool(name="oh", bufs=4) as ohp,
            tc.tile_pool(name="pso", bufs=8, space="PSUM") as pso,
        ):
            kvec_t = cpool.tile([2 * NR, 1], F32)
            nc.scalar.dma_start(kvec_t[:], kvec_d[:])
            w1_t = cpool.tile([2 * NR, 2 * NOUT], BF16)
            nc.sync.dma_start(w1_t[:], w1_d[:])
            wb_t = cpool.tile([2 * NBB, 2 * NOUT], BF16)
            nc.sync.dma_start(wb_t[:], wb_d[:])
            bias_t = cpool.tile([2 * NOUT, 1], F32)
            nc.scalar.dma_start(bias_t[:], bias_d[:])

            # superblocks of 2 chunks: one dma_start covers both chunk rows
            # (amortizes DMA fixed costs); outputs drain per chunk
            spans = []
            c = 0
            while c < n_chunks:
                s = min(2, n_chunks - c)
                spans.append((c, s))
                c += s
            for c, span in spans:
                # bb (the bigger stream) on the SP ring; catr + out on the ACT
                # ring so the two HWDGE rings generate descriptors in parallel
                catr_t = catrp.tile([2 * NR, span, free], FP8, tag="catr")
                bb_t = bbp.tile([2 * NBB, span, free], BF16, tag="bb")
                # alternate the in-streams between the rings per superblock to
                # even out per-ring byte flow over time
                ra, rb = (nc.scalar, nc.sync) if (c // 2) % 2 == 0 else (nc.sync, nc.scalar)
                ra.dma_start(catr_t[:],
                             catr_d[c:c + span].rearrange("c r a -> r c a"))
                rb.dma_start(bb_t[:],
                             bb_d[c:c + span].rearrange("c r a -> r c a"))
                out_t = outp.tile([2 * NOUT, span, free], BF16, tag="out")
                for j in range(span):
                    for p in range(pairs_per_chunk):
                        sl = bass.ts(p, G)
                        oh = ohp.tile([2 * NR, G], BF16)
                        nc.vector.tensor_scalar(oh[:], catr_t[:, j, sl],
                                                kvec_t[:], None,
                                                mybir.AluOpType.is_equal)
                        ops = pso.tile([2 * NOUT, G], F32)
                        nc.tensor.matmul(ops[:], w1_t[:], oh[:],
                                         start=True, stop=False)
                        nc.tensor.matmul(ops[:], wb_t[:], bb_t[:, j, sl],
                                         start=False, stop=True)
                        # bias-add + PSUM->SBUF copy on the ACT engine
                        nc.scalar.activation(out_t[:, j, sl], ops[:],
                                             mybir.ActivationFunctionType.Identity,
                                             bias=bias_t[:], scale=1.0)
                    # drain each finished chunk immediately (shorter pipeline
                    # tail than one superblock-sized out-DMA)
                    nc.scalar.dma_start(out_d[c + j], out_t[:, j])
    nc.compile()
    return nc


def shard_blobs(ai, core, catr_cols, n_chunks=N_CHUNKS,
                pairs_per_chunk=PAIRS_PER_CHUNK):
    """Slice core's shard and lay it out feature-major, 2-group packed."""
    free = pairs_per_chunk * G
    npad = n_chunks * pairs_per_chunk * PAIR
    shard = ai[core * N_SHARD:(core + 1) * N_SHARD]
    n_use = min(N_SHARD, npad)
    padded = np.zeros((npad, ai.shape[1]), np.float32)
    padded[:n_use] = shard[:n_use]
    # [chunk, pair, group, atom, col]
    v = padded.reshape(n_chunks, pairs_per_chunk, 2, G, ai.shape[1])
    catr = np.ascontiguousarray(v[..., catr_cols].transpose(0, 2, 4, 1, 3)
                                .reshape(n_chunks, 2 * NR, free)).astype(NPFP8)
    bb = np.ascontiguousarray(v[..., BIN16 + list(range(30, 78))]
                              .transpose(0, 2, 4, 1, 3)
                              .reshape(n_chunks, 2 * NBB, free)).astype(NPBF16)
    return catr, bb


def unshard_out(o, n_chunks=N_CHUNKS, pairs_per_chunk=PAIRS_PER_CHUNK):
    """[n_chunks, 128, free] device layout -> [npad, 64] atom-major."""
    npad = n_chunks * pairs_per_chunk * PAIR
    return (o.reshape(n_chunks, 2, NOUT, pairs_per_chunk, G)
            .transpose(0, 3, 1, 4, 2).reshape(npad, NOUT))


def _install_ntff_hook():
    """Register the axon NTFF profile hook that this image's antenv lacks."""
    import types
    try:
        import antenv.axon_hooks  # noqa: F401
        return
    except ImportError:
        pass
    try:
        from trn_agent_boot.trn_boot import _ntff_profile_via_ctypes
        hook = _ntff_profile_via_ctypes("/opt/axon/libaxon_pjrt.so")
        mod = types.ModuleType("antenv.axon_hooks")
        _state = {"hook": hook}
        mod.set_axon_ntff_profile_hook = lambda h: _state.__setitem__("hook", h)
        mod.get_axon_ntff_profile_hook = lambda: _state["hook"]
        sys.modules["antenv.axon_hooks"] = mod
        import antenv
        antenv.axon_hooks = mod
    except Exception as e:  # profiling is best-effort
        print(f"ntff hook install failed: {e}", file=sys.stderr)


def kernel(**inputs):
    consts, catr_cols = build_tables(inputs)
    ai = np.ascontiguousarray(np.asarray(inputs["atom_inputs"], dtype=np.float32))
    assert ai.shape == (N_TOTAL, 78), ai.shape

    in_maps = []
    for i in range(N_CORES):
        catr, bb = shard_blobs(ai, i, catr_cols)
        in_maps.append({"catr": catr, "bb": bb, **consts})

    trace = bool(int(os.environ.get("KERNEL_TRACE", "0")))
    if trace:
        _install_ntff_hook()
    nc = build_nc()
    res = run_bass_kernel_spmd(
        nc, in_maps, core_ids=list(range(N_CORES)), trace=trace,
    )
    kernel.last_result = res

    outs = []
    for i in range(N_CORES):
        o = np.asarray(res.results[i]["out"]).astype(np.float32)
        outs.append(unshard_out(o)[:N_SHARD])
    return np.ascontiguousarray(np.concatenate(outs, axis=0))


kernel.last_result = None


# revision 43
# speedup vs baseline: 1.0253x; 1.0027x over previous
"""Trainium2 Bass kernel for nn_AtomEmbedding (embedding_lookup, memory-bound).

Strategy (pure data parallel over 8 NeuronCores):
  - Every table lookup is expressed as one-hot(compare) @ table on the PE.
  - The multi-class feature columns are replicated host-side into one row per
    non-default class (the "catr" blob, 50 rows per atom-group), so a single
    DVE tensor_scalar(is_equal) against a per-partition constant turns the
    DMA'd tile directly into one-hot indicators - no broadcast matmul.
  - Binary (0/1) columns ARE their own one-hot: 5 of them ride in catr (their
    eq-against-1 is the identity), the other 16 feed the matmul raw, packed
    with the bond features into one 128-row contraction ("bb" blob).
  - Two accumulating matmuls per 512-atom group pair (PE runs at the cold
    1.2 GHz clock on this part, so streamed columns are the scarce resource):
      W1'^T @ onehot[100,512] + Wb^T @ bb[128,512] -> psum[128,512].
  - Table edge semantics (element LUT default, ringsize unknown->6, ring clip)
    fold EXACTLY into weights via a delta trick + bias vector applied by the
    Scalar engine during the PSUM->SBUF copy.
  - Everything is packed 2 atom-groups deep so compute ops and DMAs use up to
    128 partitions; all streams are bf16 (integer features are exact in bf16;
    folded tables round, ~4e-3 max rel err end-to-end).
  - Host-side work is sharding + column gather/replication + transpose only.
"""

import os
import sys

sys.path.insert(0, "/opt/trn_rl_repo")
os.environ.setdefault("MYCRO_LOCAL_CACHE", "1")

import ml_dtypes
import numpy as np

import concourse.bacc as bacc
import concourse.bass as bass
import concourse.mybir as mybir
import concourse.tile as tile
from concourse.bass_utils import run_bass_kernel_spmd

F32 = mybir.dt.float32
BF16 = mybir.dt.bfloat16
NPBF16 = ml_dtypes.bfloat16
FP8 = mybir.dt.float8e4
NPFP8 = ml_dtypes.float8_e4m3

N_CORES = 8
N_TOTAL = 1_500_000
N_SHARD = N_TOTAL // N_CORES  # 187500
G = 512                       # atoms per group (one matmul free dim)
PAIR = 2 * G                  # atoms per group-pair (packed 2-deep on partitions)
N_PAIRS = 184                 # pairs per core after padding
NP = N_PAIRS * PAIR           # 188416 padded atoms per core
PAIRS_PER_CHUNK = 8
N_CHUNKS = N_PAIRS // PAIRS_PER_CHUNK  # 23
FREE = PAIRS_PER_CHUNK * G    # 4096 free-dim per chunk tile

# original 78-feature column roles
MULTI_COLS = [0, 1, 2, 3, 6, 27, 28, 29]  # z, deg, chg, hyb, hc, rs, aron, fused
# binary cols moved into the one-hot tile (eq(x,1) == x): arom, don, acc, f16, f17
MOVED_BIN = [4, 25, 26, 23, 24]
# binary cols packed with bond into the "bb" matmul: f0..f15
BIN16 = list(range(7, 23))
NBOND = 48
NOH = 45                      # one-hot class rows per group (before moved bins)
NR = NOH + len(MOVED_BIN)     # 50 catr rows per group
NBB = len(BIN16) + NBOND      # 64 bb rows per group
NOUT = 64


def build_tables(inputs):
    """Fold all embedding tables + linear weights into the device constants.

    Returns (consts dict, catr_cols): catr_cols[r] = source column of the
    original 78-wide input for replicated row r (0..NR-1).
    """
    g = {k: np.asarray(v) for k, v in inputs.items()}
    elut = g["element_lut"].astype(np.int64)
    rvals = g["ring_values"].astype(np.int64)
    e_emb = g["element_embed"].astype(np.float64)
    d_emb = g["degree_embed"].astype(np.float64)
    r_emb = g["ring_embed"].astype(np.float64)
    c_emb = g["charge_embed"].astype(np.float64)
    a_emb = g["aromatic_embed"].astype(np.float64)
    hy_emb = g["hybrid_embed"].astype(np.float64)
    hc_emb = g["hydrogen_embed"].astype(np.float64)
    ft = g["func_tables"].astype(np.float64)
    don_emb = g["h_don_embed"].astype(np.float64)
    acc_emb = g["h_acc_embed"].astype(np.float64)
    rs_emb = g["ringsize_embed"].astype(np.float64)
    an_emb = g["aroma_num_embed"].astype(np.float64)
    fi_emb = g["fused_if_embed"].astype(np.float64)
    frw = g["func_reduce_w"].astype(np.float64)
    frb = g["func_reduce_b"].astype(np.float64)
    bew = g["bond_env_w"].astype(np.float64)
    beb = g["bond_env_b"].astype(np.float64)

    def func_delta(j):
        Rj = frw[:, 2 * j:2 * j + 2]
        return (ft[j, 1] - ft[j, 0]) @ Rj.T

    # replicated one-hot rows: (source col, compare value, w_row[64])
    rows = []

    def add(col, v, cols, w):
        wr = np.zeros(NOUT)
        wr[cols[0]:cols[1]] = w
        rows.append((col, float(v), wr))

    e_def = int(np.clip(elut[0], 0, 6))
    for v in range(1, 17):
        idx = int(np.clip(elut[v], 0, 6))
        if idx != e_def:
            add(0, v, (0, 4), e_emb[idx] - e_emb[e_def])
    for k in range(1, 7):
        add(1, k, (4, 8), d_emb[k] - d_emb[0])
    for k in range(1, 8):
        add(2, k, (12, 16), c_emb[k] - c_emb[0])
    for k in range(1, 6):
        add(3, k, (20, 24), hy_emb[k] - hy_emb[0])
    for k in range(1, 5):
        add(6, k, (24, 28), hc_emb[k] - hc_emb[0])
    seen = set()
    for i in range(7):
        v = int(rvals[i])
        if v in seen:
            continue
        seen.add(v)
        w = rs_emb[i] - rs_emb[6]
        if np.any(w != 0.0):
            add(27, v, (36, 40), w)
    for k in range(1, 5):
        add(28, k, (40, 44), an_emb[k] - an_emb[0])
    for k in range(1, 8):
        add(29, k, (44, 48), fi_emb[k] - fi_emb[0])
    assert len(rows) == NOH, f"expected {NOH} one-hot rows, got {len(rows)}"

    # moved binary rows: compare value 1 -> indicator == raw value
    add(4, 1, (16, 20), a_emb[1] - a_emb[0])        # aromatic
    add(25, 1, (32, 34), don_emb[1] - don_emb[0])   # h_don
    add(26, 1, (34, 36), acc_emb[1] - acc_emb[0])   # h_acc
    add(23, 1, (28, 32), func_delta(16))            # f16
    add(24, 1, (28, 32), func_delta(17))            # f17
    assert len(rows) == NR

    catr_cols = [c for c, _, _ in rows]
    kvec = np.array([v for _, v, _ in rows])
    W1 = np.stack([w for _, _, w in rows])          # [50, 64]

    # bb-side weights: 16 func binaries then bond
    Wb1 = np.zeros((NBB, NOUT))
    for j in range(16):
        Wb1[j, 28:32] = func_delta(j)
    Wb1[16:, 48:64] = bew.T

    bias = np.zeros(NOUT)
    bias[0:4] = e_emb[e_def]
    bias[4:8] = d_emb[0]
    bias[8:12] = r_emb[1]       # ring col: clip(ring+1,0,1) == 1 always
    bias[12:16] = c_emb[0]
    bias[16:20] = a_emb[0]
    bias[20:24] = hy_emb[0]
    bias[24:28] = hc_emb[0]
    bias[28:32] = frb + sum(ft[j, 0] @ frw[:, 2 * j:2 * j + 2].T for j in range(18))
    bias[32:34] = don_emb[0]
    bias[34:36] = acc_emb[0]
    bias[36:40] = rs_emb[6]
    bias[40:44] = an_emb[0]
    bias[44:48] = fi_emb[0]
    bias[48:64] = beb

    # 2-group block-diagonal packing (bf16 matmul operands)
    kvec2 = np.tile(kvec, 2).reshape(2 * NR, 1).astype(np.float32)
    w1b = np.zeros((2 * NR, 2 * NOUT), NPBF16)
    w1b[0:NR, 0:NOUT] = W1
    w1b[NR:, NOUT:] = W1
    wbb = np.zeros((2 * NBB, 2 * NOUT), NPBF16)
    wbb[0:NBB, 0:NOUT] = Wb1
    wbb[NBB:, NOUT:] = Wb1
    bias2 = np.tile(bias, 2).reshape(2 * NOUT, 1).astype(np.float32)

    consts = {"kvec": kvec2, "w1": w1b, "wb": wbb, "bias": bias2}
    return consts, catr_cols


def build_nc(n_chunks=N_CHUNKS, pairs_per_chunk=PAIRS_PER_CHUNK):
    free = pairs_per_chunk * G
    # Bacc (not plain Bass): its compile() runs move_matmul_waits_to_ldweights
    # + generate_event_semaphores, which legalize to walrus's 1-wait-per-
    # instruction constraint on TRN2.
    nc = bacc.Bacc(None)
    catr_d = nc.dram_tensor("catr", [n_chunks, 2 * NR, free], FP8,
                            kind="ExternalInput")
    bb_d = nc.dram_tensor("bb", [n_chunks, 2 * NBB, free], BF16,
                          kind="ExternalInput")
    kvec_d = nc.dram_tensor("kvec", [2 * NR, 1], F32, kind="ExternalInput")
    w1_d = nc.dram_tensor("w1", [2 * NR, 2 * NOUT], BF16, kind="ExternalInput")
    wb_d = nc.dram_tensor("wb", [2 * NBB, 2 * NOUT], BF16, kind="ExternalInput")
    bias_d = nc.dram_tensor("bias", [2 * NOUT, 1], F32, kind="ExternalInput")
    out_d = nc.dram_tensor("out", [n_chunks, 2 * NOUT, free], BF16,
                           kind="ExternalOutput")

    with tile.TileContext(nc) as tc:
        with (
            tc.tile_pool(name="consts", bufs=1) as cpool,
            tc.tile_pool(name="catr", bufs=5) as catrp,
            tc.tile_pool(name="bb", bufs=4) as bbp,
            tc.tile_pool(name="outs", bufs=4) as outp,
            tc.tile_pool(name="oh", bufs=4) as ohp,
            tc.tile_pool(name="pso", bufs=8, space="PSUM") as pso,
        ):
            kvec_t = cpool.tile([2 * NR, 1], F32)
            nc.scalar.dma_start(kvec_t[:], kvec_d[:])
            w1_t = cpool.tile([2 * NR, 2 * NOUT], BF16)
            nc.sync.dma_start(w1_t[:], w1_d[:])
            wb_t = cpool.tile([2 * NBB, 2 * NOUT], BF16)
            nc.sync.dma_start(wb_t[:], wb_d[:])
            bias_t = cpool.tile([2 * NOUT, 1], F32)
            nc.scalar.dma_start(bias_t[:], bias_d[:])

            # superblocks of 2 chunks: one dma_start covers both chunk rows
            # (amortizes DMA fixed costs); outputs drain per chunk
            spans = []
            c = 0
            while c < n_chunks:
                s = min(2, n_chunks - c)
                spans.append((c, s))
                c += s
            for c, span in spans:
                # bb (the bigger stream) on the SP ring; catr + out on the ACT
                # ring so the two HWDGE rings generate descriptors in parallel
                catr_t = catrp.tile([2 * NR, span, free], FP8, tag="catr")
                bb_t = bbp.tile([2 * NBB, span, free], BF16, tag="bb")
                # alternate the in-streams between the rings per superblock to
                # even out per-ring byte flow over time
                ra, rb = (nc.scalar, nc.sync) if (c // 2) % 2 == 0 else (nc.sync, nc.scalar)
                ra.dma_start(catr_t[:],
                             catr_d[c:c + span].rearrange("c r a -> r c a"))
                rb.dma_start(bb_t[:],
                             bb_d[c:c + span].rearrange("c r a -> r c a"))
                out_t = outp.tile([2 * NOUT, span, free], BF16, tag="out")
                for j in range(span):
                    for p in range(pairs_per_chunk):
                        sl = bass.ts(p, G)
                        oh = ohp.tile([2 * NR, G], BF16)
                        nc.vector.tensor_scalar(oh[:], catr_t[:, j, sl],
                                                kvec_t[:], None,
                                                mybir.AluOpType.is_equal)
                        ops = pso.tile([2 * NOUT, G], F32)
                        nc.tensor.matmul(ops[:], w1_t[:], oh[:],
                                         start=True, stop=False)
                        nc.tensor.matmul(ops[:], wb_t[:], bb_t[:, j, sl],
                                         start=False, stop=True)
                        # bias-add + PSUM->SBUF copy on the ACT engine
                        nc.scalar.activation(out_t[:, j, sl], ops[:],
                                             mybir.ActivationFunctionType.Identity,
                                             bias=bias_t[:], scale=1.0)
                    # drain each finished chunk immediately (shorter pipeline
                    # tail than one superblock-sized out-DMA), alternating
                    # rings per chunk
                    ro = nc.scalar if (c + j) % 2 == 0 else nc.sync
                    ro.dma_start(out_d[c + j], out_t[:, j])
    nc.compile()
    return nc


def shard_blobs(ai, core, catr_cols, n_chunks=N_CHUNKS,
                pairs_per_chunk=PAIRS_PER_CHUNK):
    """Slice core's shard and lay it out feature-major, 2-group packed."""
    free = pairs_per_chunk * G
    npad = n_chunks * pairs_per_chunk * PAIR
    shard = ai[core * N_SHARD:(core + 1) * N_SHARD]
    n_use = min(N_SHARD, npad)
    padded = np.zeros((npad, ai.shape[1]), np.float32)
    padded[:n_use] = shard[:n_use]
    # [chunk, pair, group, atom, col]
    v = padded.reshape(n_chunks, pairs_per_chunk, 2, G, ai.shape[1])
    catr = np.ascontiguousarray(v[..., catr_cols].transpose(0, 2, 4, 1, 3)
                                .reshape(n_chunks, 2 * NR, free)).astype(NPFP8)
    bb = np.ascontiguousarray(v[..., BIN16 + list(range(30, 78))]
                              .transpose(0, 2, 4, 1, 3)
                              .reshape(n_chunks, 2 * NBB, free)).astype(NPBF16)
    return catr, bb


def unshard_out(o, n_chunks=N_CHUNKS, pairs_per_chunk=PAIRS_PER_CHUNK):
    """[n_chunks, 128, free] device layout -> [npad, 64] atom-major."""
    npad = n_chunks * pairs_per_chunk * PAIR
    return (o.reshape(n_chunks, 2, NOUT, pairs_per_chunk, G)
            .transpose(0, 3, 1, 4, 2).reshape(npad, NOUT))


def _install_ntff_hook():
    """Register the axon NTFF profile hook that this image's antenv lacks."""
    import types
    try:
        import antenv.axon_hooks  # noqa: F401
        return
    except ImportError:
        pass
    try:
        from trn_agent_boot.trn_boot import _ntff_profile_via_ctypes
        hook = _ntff_profile_via_ctypes("/opt/axon/libaxon_pjrt.so")
        mod = types.ModuleType("antenv.axon_hooks")
        _state = {"hook": hook}
        mod.set_axon_ntff_profile_hook = lambda h: _state.__setitem__("hook", h)
        mod.get_axon_ntff_profile_hook = lambda: _state["hook"]
        sys.modules["antenv.axon_hooks"] = mod
        import antenv
        antenv.axon_hooks = mod
    except Exception as e:  # profiling is best-effort
        print(f"ntff hook install failed: {e}", file=sys.stderr)


def kernel(**inputs):
    consts, catr_cols = build_tables(inputs)
    ai = np.ascontiguousarray(np.asarray(inputs["atom_inputs"], dtype=np.float32))
    assert ai.shape == (N_TOTAL, 78), ai.shape

    in_maps = []
    for i in range(N_CORES):
        catr, bb = shard_blobs(ai, i, catr_cols)
        in_maps.append({"catr": catr, "bb": bb, **consts})

    trace = bool(int(os.environ.get("KERNEL_TRACE", "0")))
    if trace:
        _install_ntff_hook()
    nc = build_nc()
    res = run_bass_kernel_spmd(
        nc, in_maps, core_ids=list(range(N_CORES)), trace=trace,
    )
    kernel.last_result = res

    outs = []
    for i in range(N_CORES):
        o = np.asarray(res.results[i]["out"]).astype(np.float32)
        outs.append(unshard_out(o)[:N_SHARD])
    return np.ascontiguousarray(np.concatenate(outs, axis=0))


kernel.last_result = None
